# revision 1
# baseline (speedup 1.0000x reference)
"""LocalCorrelation (13x13 cost volume) Trainium2 kernel.

Full inputs z_t, z_t1: [8, 256, 128, 128] f32 -> out [8, 169, 128, 128] f32.
out[b, 13*di+dj, h, w] = sum_c z_t[b,c,h,w] * pad(z_t1)[b,c,h+di,w+dj] / 16

Sharding: data-parallel over batch, 1 batch element per NeuronCore (8 cores).

Per-core algorithm (SPMD, identical program):
  - Load z_t (scaled by 1/16) and zero-padded z_t1 into SBUF as bf16,
    channel dim on partitions (2 chunks of 128).
  - For each 8x16 output-pixel block: TensorE "block gram" matmuls
    stationary = z_t block [c,128 pixels], streaming = padded z_t1
    20x28 window [c,560] -> PSUM f32 (accumulated over 2 c-chunks).
  - PSUM -> SBUF bf16, dense DMA to DRAM scratch.
  - Shear-gather DMAs (per di) read the 13x13 tap band back into
    [di*8+dh, (w, dj)] layout -- the per-pixel diagonal offset is
    absorbed by flat DRAM addressing.
  - On-chip strided copy transposes (w,dj)->(dj,w) and casts to f32.
  - Output DMA writes [tap][h][w] with 512B runs.
"""

import numpy as np

C = 256
H = W = 128
KS = 13
KK = 169
RAD = 6
HP = WP = 140  # padded spatial
SA = 8  # block rows (stripe height)
SB = 16  # block cols
NWB = W // SB  # 8 w-blocks per stripe
NST = H // SA  # 16 stripes
WINP = SA + 2 * RAD  # 20 streamed rows per window
WINQ = SB + 2 * RAD  # 28 streamed cols per window
WIN = WINP * WINQ  # 560

_cache = {}


def _build():
    import concourse.bass as bass
    import concourse.mybir as mybir
    import concourse.tile as tile
    from concourse import bacc

    f32 = mybir.dt.float32
    bf16 = mybir.dt.bfloat16

    nc = bacc.Bacc("TRN2", target_bir_lowering=False, debug=False)
    zt_d = nc.dram_tensor("z_t", [C, H, W], f32, kind="ExternalInput")
    z1_d = nc.dram_tensor("z_t1", [C, H, W], f32, kind="ExternalInput")
    out_d = nc.dram_tensor("out", [KK, H, W], f32, kind="ExternalOutput")

    with tile.TileContext(nc) as tc:
        with tc.tile_pool(name="persist", bufs=1) as pp:
            ZT = [pp.tile([128, H * W], bf16, tag=f"zt{k}", name=f"zt{k}") for k in range(2)]
            Z1P = [pp.tile([128, HP * WP], bf16, tag=f"z1p{k}", name=f"z1p{k}") for k in range(2)]

            # ---- input load: cast f32->bf16 via SWDGE DMA ----
            # ZT is stored BLOCK-MAJOR: free index = ((si*8 + wb)*8 + dh)*16 + dw
            # so each 8x16 block's 128 pixels are contiguous (matmul stationary
            # operand requires a single free dim).
            for k in range(2):
                nc.vector.memset(Z1P[k][:, :], 0.0)

            with tc.tile_pool(name="ld", bufs=2) as ldp:
                for k in range(2):
                    for s in range(4):  # 32-row slabs
                        z1u = ldp.tile([128, 32 * W], bf16, tag="z1u", name="z1u")
                        src = z1_d.ap()[k * 128:(k + 1) * 128, s * 32:(s + 1) * 32, :]
                        nc.gpsimd.dma_start(
                            z1u.rearrange("c (h w) -> c h w", h=32), src)
                        dst = Z1P[k].rearrange("c (h w) -> c h w", h=HP)[
                            :, RAD + s * 32: RAD + (s + 1) * 32, RAD: RAD + W]
                        nc.vector.tensor_copy(dst, z1u.rearrange("c (h w) -> c h w", h=32))
                for k in range(2):
                    for s in range(4):  # 32-row slabs -> 4 stripes each
                        ztu = ldp.tile([128, 32 * W], bf16, tag="ztu", name="ztu")
                        src = zt_d.ap()[k * 128:(k + 1) * 128, s * 32:(s + 1) * 32, :]
                        nc.gpsimd.dma_start(
                            ztu.rearrange("c (h w) -> c h w", h=32), src)
                        for sl in range(4):
                            si_g = s * 4 + sl
                            srcv = ztu.rearrange(
                                "c (h wb dw) -> c wb h dw", h=32, wb=NWB)[
                                :, :, sl * SA:(sl + 1) * SA, :]
                            dstv = ZT[k][:, si_g * 1024:(si_g + 1) * 1024].rearrange(
                                "c (wb dh dw) -> c wb dh dw", wb=NWB, dh=SA)
                            nc.vector.tensor_copy(dstv, srcv)
            for k in range(2):
                nc.vector.tensor_scalar_mul(ZT[k][:, :], ZT[k][:, :], 1.0 / 16.0)

            # ---- main loop ----
            with (
                tc.tile_pool(name="xbp", bufs=2) as xbp,
                tc.tile_pool(name="o2p", bufs=2) as o2p,
                tc.tile_pool(name="o3p", bufs=2) as o3p,
                tc.tile_pool(name="psp", bufs=2, space="PSUM") as psp,
                tc.tile_pool(name="scrp", bufs=2, space="DRAM") as scrp,
            ):
                for si in range(NST):
                    h0 = si * SA
                    scr = scrp.tile([NWB, 128, WIN], bf16, tag="scr", name="scr")
                    xb = xbp.tile([128, NWB * WIN], bf16, tag="xb", name="xb")
                    for wb in range(NWB):
                        w0 = wb * SB
                        ps = [psp.tile([128, 280], f32, tag=f"ps{i}", name=f"ps{i}")
                              for i in range(2)]
                        for k in range(2):
                            blk = si * NWB + wb
                            lhsT = ZT[k][:, blk * 128:(blk + 1) * 128]
                            for half in range(2):
                                rhs = Z1P[k].rearrange("c (h w) -> c h w", h=HP)[
                                    :, h0 + 10 * half: h0 + 10 * (half + 1),
                                    w0:w0 + WINQ]
                                nc.tensor.matmul(ps[half][:, :], lhsT, rhs,
                                                 start=(k == 0), stop=(k == 1))
                        for half in range(2):
                            dst = xb[:, wb * WIN + half * 280: wb * WIN + (half + 1) * 280]
                            if wb % 2 == 0:
                                nc.scalar.copy(dst, ps[half][:, :])
                            else:
                                nc.vector.tensor_copy(dst, ps[half][:, :])

                    # dense scratch write (1120B runs per (m, wb))
                    scr_w = bass.AP(scr.tensor, 0, [[WIN, 128], [128 * WIN, NWB], [1, WIN]])
                    nc.sync.dma_start(scr_w, xb.rearrange("p (wb s) -> p wb s", wb=NWB))

                    # shear-gather: per (di, wb), absorb diagonal in DRAM strides
                    # (DMA APs are limited to 3 dims)
                    o2 = o2p.tile([104, 128 * KS], bf16, tag="o2", name="o2")
                    for di in range(KS):
                        for wb in range(NWB):
                            src = bass.AP(scr.tensor, di * WINQ + wb * 128 * WIN,
                                          [[SB * WIN + WINQ, SA],
                                           [WIN + 1, SB],
                                           [1, KS]])
                            dst = o2[di * SA:(di + 1) * SA,
                                     wb * SB * KS:(wb + 1) * SB * KS].rearrange(
                                "p (dw dj) -> p dw dj", dw=SB)
                            nc.sync.dma_start(dst, src)

                    # (w, dj) -> (dj, w) transpose + cast to f32
                    o3 = o3p.tile([104, KS * W], f32, tag="o3", name="o3")
                    src_t = o2.rearrange("p (w dj) -> p dj w", dj=KS)
                    dst_t = o3.rearrange("p (dj w) -> p dj w", dj=KS)
                    if si % 2 == 0:
                        nc.vector.tensor_copy(dst_t, src_t)
                    else:
                        nc.scalar.copy(dst_t, src_t)

                    # final output write: 512B runs
                    for di in range(KS):
                        srcw = o3[di * SA:(di + 1) * SA, :].rearrange(
                            "p (dj w) -> p dj w", dj=KS)
                        dstw = bass.AP(out_d, di * KS * H * W + h0 * W,
                                       [[W, SA], [H * W, KS], [1, W]])
                        nc.sync.dma_start(dstw, srcw)

    nc.compile()
    return nc


def _get_nc():
    if "nc" not in _cache:
        _cache["nc"] = _build()
    return _cache["nc"]


def kernel(z_t: np.ndarray, z_t1: np.ndarray) -> np.ndarray:
    from concourse.bass_utils import run_bass_kernel_spmd

    nc = _get_nc()
    z_t = np.ascontiguousarray(z_t, dtype=np.float32)
    z_t1 = np.ascontiguousarray(z_t1, dtype=np.float32)
    B = z_t.shape[0]
    in_maps = [{"z_t": z_t[i], "z_t1": z_t1[i]} for i in range(B)]
    res = run_bass_kernel_spmd(nc, in_maps, core_ids=list(range(B)))
    return np.stack([res.results[i]["out"] for i in range(B)], axis=0)



# revision 3
# speedup vs baseline: 4.1152x; 4.1152x over previous
"""LocalCorrelation (13x13 cost volume) Trainium2 kernel.

Full inputs z_t, z_t1: [8, 256, 128, 128] f32 -> out [8, 169, 128, 128] f32.
out[b, 13*di+dj, h, w] = sum_c z_t[b,c,h,w] * pad(z_t1)[b,c,h+di,w+dj] / 16

Sharding: data-parallel over batch, 1 batch element per NeuronCore (8 cores).
Host casts inputs to bf16 and upcasts the bf16 output back to f32.

Per-core algorithm (SPMD, identical program):
  - z_t (block-major) and zero-padded z_t1 resident in SBUF as bf16,
    channel dim on partitions (2 chunks of 128).
  - For each 8x16 output-pixel block: TensorE block-gram matmuls
    (stationary = z_t block [c,128 pix], streaming = padded z_t1 20x28
    window in two 280-col halves) -> PSUM f32, accumulated over c-chunks.
  - PSUM -> SBUF evacuation with the 1/16 scale folded in (ACT + DVE).
  - One dense DMA per stripe writes xb to a DRAM scratch laid out as
      addr = 4487*pix + 561*wb + (28*p + q) + 574560*si
    chosen so that the 4-dim shear gather collapses to THREE AP dims:
      dh-stride 16*4487+28 = 71820, si-stride 8*71820 (partition dim
      (si,dh) = h is linear), (dw,wb)-stride 561 (merged, dw-major),
      dj contiguous runs of 13.
  - 13 gather DMAs (one per di) read the whole image's tap rows:
      [[71820,128],[561,128],[1,13]] -> o2[h, (dw,wb,dj)].
  - One DVE permute copy per di -> o4[h, (dj, w)].
  - One output DMA per di: out[di*13+dj, h, w] with 256B w-runs.
"""

import numpy as np

C = 256
H = W = 128
KS = 13
KK = 169
RAD = 6
HP = WP = 140  # padded spatial
SA = 8   # block rows
SB = 16  # block cols
NWB = W // SB   # 8 w-blocks per stripe
NST = H // SA   # 16 stripes
WINP = SA + 2 * RAD   # 20 streamed rows per window
WINQ = SB + 2 * RAD   # 28 streamed cols per window
WIN = WINP * WINQ     # 560

# scratch layout strides (elements)
PIXS = 4487            # pixel slot stride (> 7*561+559, coprime-ish)
WBS = 561              # wb slot stride (win extent 560 + 1)
DHS = 16 * PIXS + 28   # 71820: dh stride seen by the gather
SIS = 8 * DHS          # 574560: per-stripe stride (makes (si,dh) linear)
XBW = 8 * WBS          # 4488: xb tile width
ODW = 128 * KS         # 1664: o2/o4 tile width

_cache = {}


def _build():
    import concourse.bass as bass
    import concourse.mybir as mybir
    import concourse.tile as tile
    from concourse import bacc

    f32 = mybir.dt.float32
    bf16 = mybir.dt.bfloat16

    nc = bacc.Bacc("TRN2", target_bir_lowering=False, debug=False)
    zt_d = nc.dram_tensor("z_t", [C, H, W], bf16, kind="ExternalInput")
    z1_d = nc.dram_tensor("z_t1", [C, H, W], bf16, kind="ExternalInput")
    out_d = nc.dram_tensor("out", [KK, H, W], bf16, kind="ExternalOutput")

    with tile.TileContext(nc) as tc:
        with tc.tile_pool(name="persist", bufs=1) as pp:
            ZT = [pp.tile([128, H * W], bf16, tag=f"zt{k}", name=f"zt{k}") for k in range(2)]
            Z1P = [pp.tile([128, HP * WP], bf16, tag=f"z1p{k}", name=f"z1p{k}") for k in range(2)]

            for k in range(2):
                nc.vector.memset(Z1P[k][:, :], 0.0)

            # ---- input loads ----
            # z_t1: direct DMA into the padded interior (no staging copy)
            for k in range(2):
                for s in range(4):  # 32-row slabs
                    dst = Z1P[k].rearrange("c (h w) -> c h w", h=HP)[
                        :, RAD + s * 32: RAD + (s + 1) * 32, RAD: RAD + W]
                    src = z1_d.ap()[k * 128:(k + 1) * 128, s * 32:(s + 1) * 32, :]
                    eng = nc.sync if s % 2 == 0 else nc.scalar
                    eng.dma_start(dst, src)
            # z_t: stage slabs, rearrange to block-major
            # ZT free index = ((si*8 + wb)*8 + dh)*16 + dw
            with tc.tile_pool(name="ld", bufs=2) as ldp:
                for k in range(2):
                    for s in range(4):
                        ztu = ldp.tile([128, 32 * W], bf16, tag="ztu", name="ztu")
                        src = zt_d.ap()[k * 128:(k + 1) * 128, s * 32:(s + 1) * 32, :]
                        nc.sync.dma_start(
                            ztu.rearrange("c (h w) -> c h w", h=32), src)
                        for sl in range(4):
                            si_g = s * 4 + sl
                            srcv = ztu.rearrange(
                                "c (h wb dw) -> c wb h dw", h=32, wb=NWB)[
                                :, :, sl * SA:(sl + 1) * SA, :]
                            dstv = ZT[k][:, si_g * 1024:(si_g + 1) * 1024].rearrange(
                                "c (wb dh dw) -> c wb dh dw", wb=NWB, dh=SA)
                            if sl % 2 == 0:
                                nc.vector.tensor_copy(dstv, srcv)
                            else:
                                nc.scalar.copy(dstv, srcv)

            # ---- main pipeline ----
            with (
                tc.tile_pool(name="xbp", bufs=2) as xbp,
                tc.tile_pool(name="o2p", bufs=3) as o2p,
                tc.tile_pool(name="o4p", bufs=2) as o4p,
                tc.tile_pool(name="psp", bufs=2, space="PSUM") as psp,
                tc.tile_pool(name="scrp", bufs=1, space="DRAM") as scrp,
            ):
                scr = scrp.tile([NST, SIS], bf16, tag="scr", name="scr")
                for si in range(NST):
                    h0 = si * SA
                    xb = xbp.tile([128, XBW], bf16, tag="xb", name="xb")
                    for wb in range(NWB):
                        w0 = wb * SB
                        ps = [psp.tile([128, 280], f32, tag=f"ps{i}", name=f"ps{i}")
                              for i in range(2)]
                        for k in range(2):
                            blk = si * NWB + wb
                            lhsT = ZT[k][:, blk * 128:(blk + 1) * 128]
                            for half in range(2):
                                rhs = Z1P[k].rearrange("c (h w) -> c h w", h=HP)[
                                    :, h0 + 10 * half: h0 + 10 * (half + 1),
                                    w0:w0 + WINQ]
                                nc.tensor.matmul(ps[half][:, :], lhsT, rhs,
                                                 start=(k == 0), stop=(k == 1))
                        # evacuate PSUM -> xb with the 1/sqrt(C) scale folded in
                        for half in range(2):
                            dst = xb[:, wb * WBS + half * 280: wb * WBS + half * 280 + 280]
                            if half == 0:
                                nc.scalar.mul(dst, ps[half][:, :], 1.0 / 16.0)
                            else:
                                nc.vector.tensor_scalar_mul(dst, ps[half][:, :], 1.0 / 16.0)

                    # dense scratch write (1120B runs), SWDGE queue
                    w_src = xb[:, :].rearrange("p (wb w) -> p wb w", wb=NWB)[:, :, 0:WIN]
                    w_dst = bass.AP(scr.tensor, si * SIS, [[PIXS, 128], [WBS, NWB], [1, WIN]])
                    nc.gpsimd.dma_start(w_dst, w_src)

                # shear gather: 13 DMAs for the whole image
                for di in range(KS):
                    o2 = o2p.tile([128, ODW], bf16, tag="o2", name="o2")
                    rd_src = bass.AP(scr.tensor, 28 * di, [[DHS, 128], [WBS, 128], [1, KS]])
                    rd_dst = o2[:, :].rearrange("p (w dj) -> p w dj", dj=KS)
                    nc.sync.dma_start(rd_dst, rd_src)
                    # free-dim permute (dw,wb,dj) -> (dj,wb,dw) so w is contiguous
                    o4 = o4p.tile([128, ODW], bf16, tag="o4", name="o4")
                    csrc = o2[:, :].rearrange("p (dw wb dj) -> p dj wb dw", dw=SB, wb=NWB)
                    cdst = o4[:, :].rearrange("p (dj wb dw) -> p dj wb dw", dj=KS, wb=NWB)
                    nc.vector.tensor_copy(cdst, csrc)
                    # final output write: 256B w-runs
                    ow_dst = bass.AP(out_d, di * KS * H * W, [[W, 128], [H * W, KS], [1, W]])
                    ow_src = o4[:, :].rearrange("p (dj w) -> p dj w", dj=KS)
                    nc.scalar.dma_start(ow_dst, ow_src)

    nc.compile()
    return nc


def _get_nc():
    if "nc" not in _cache:
        _cache["nc"] = _build()
    return _cache["nc"]


def kernel(z_t: np.ndarray, z_t1: np.ndarray) -> np.ndarray:
    import ml_dtypes
    from concourse.bass_utils import run_bass_kernel_spmd

    nc = _get_nc()
    bf = ml_dtypes.bfloat16
    z_t = np.ascontiguousarray(z_t).astype(bf)
    z_t1 = np.ascontiguousarray(z_t1).astype(bf)
    B = z_t.shape[0]
    in_maps = [{"z_t": z_t[i], "z_t1": z_t1[i]} for i in range(B)]
    res = run_bass_kernel_spmd(nc, in_maps, core_ids=list(range(B)))
    return np.stack(
        [np.asarray(res.results[i]["out"]).astype(np.float32) for i in range(B)],
        axis=0)


# revision 4
# speedup vs baseline: 4.8147x; 1.1700x over previous
"""LocalCorrelation (13x13 cost volume) Trainium2 kernel.

Full inputs z_t, z_t1: [8, 256, 128, 128] f32 -> out [8, 169, 128, 128] f32.
out[b, 13*di+dj, h, w] = sum_c z_t[b,c,h,w] * pad(z_t1)[b,c,h+di,w+dj] / 16

Sharding: data-parallel over batch, 1 batch element per NeuronCore (8 cores).
Host pre-processing (free): cast to bf16, block-major reorder of z_t,
zero-pad of z_t1.  Host post-processing: upcast bf16 output to f32.

Per-core algorithm:
  - 4 big input DMAs (z_t block-major, z_t1 padded), both c-chunks.
  - Per 8x16 pixel block: block-gram matmuls, stationary = z_t block
    [c, 128 pix], streaming = padded z_t1 20x28 window in two 280-col
    halves -> one [128,1024] PSUM tile (halves at cols 0 and 512),
    accumulated over the 2 c-chunks.
  - ONE fused PSUM->SBUF evacuation per block with the 1/16 scale
    (ACT/DVE alternating) -> xb[pix, wb*560 + win], win = 28*p + q.
  - ONE dense 2-dim DMA per stripe -> DRAM scratch:
      addr = 573664*si + 4480*pix + 560*wb + win   (fully dense)
  - Superset gather, 32 DMAs with 728B runs (16K descriptors total):
      o2[h, (dw, wb', s)] = scr[71708*h + 4480*dw + 560*(wb'+4*wh) + s]
    Because 71708 = 16*4480 + 28, the per-h read start absorbs the
    diagonal: s = 28*di + dw + dj.
  - The dj-shear is now affine in FREE dims only: one 4-dim strided
    copy per (wh, di) extracts o4[h, (dj, wb', dw)], and one DMA per
    (wh, di) writes out[di*13+dj, h, w] with 128B w-runs.
"""

import numpy as np

C = 256
H = W = 128
KS = 13
KK = 169
RAD = 6
HP = WP = 140
SA = 8
SB = 16
NWB = 8
NST = 16
WINQ = 28
WIN = 560            # 20 * 28
PIXS = 4480          # scratch pixel stride (= 8 * 560, dense)
WBS = 560
DHS = 16 * PIXS + 28   # 71708
SIS = 8 * DHS          # 573664
XBW = 8 * WBS          # 4480
RUN = 364              # 13 * 28 superset run
O2W = 16 * 4 * RUN     # 23296
O4W = KS * 64          # 832

_cache = {}


def _build():
    import concourse.bass as bass
    import concourse.mybir as mybir
    import concourse.tile as tile
    from concourse import bacc

    f32 = mybir.dt.float32
    bf16 = mybir.dt.bfloat16

    nc = bacc.Bacc("TRN2", target_bir_lowering=False, debug=False)
    zt_d = nc.dram_tensor("zt", [2, 128, H * W], bf16, kind="ExternalInput")
    z1_d = nc.dram_tensor("z1p", [2, 128, HP * WP], bf16, kind="ExternalInput")
    out_d = nc.dram_tensor("out", [KK, H, W], bf16, kind="ExternalOutput")

    with tile.TileContext(nc) as tc:
        with tc.tile_pool(name="persist", bufs=1) as pp:
            ZT = [pp.tile([128, H * W], bf16, tag=f"zt{k}", name=f"zt{k}") for k in range(2)]
            Z1P = [pp.tile([128, HP * WP], bf16, tag=f"z1p{k}", name=f"z1p{k}") for k in range(2)]

            # interleave loads so k=0 matmuls can start early
            nc.sync.dma_start(Z1P[0][:, :], z1_d.ap()[0])
            nc.scalar.dma_start(ZT[0][:, :], zt_d.ap()[0])
            nc.sync.dma_start(Z1P[1][:, :], z1_d.ap()[1])
            nc.scalar.dma_start(ZT[1][:, :], zt_d.ap()[1])

            with (
                tc.tile_pool(name="xbp", bufs=1) as xbp,
                tc.tile_pool(name="o2p", bufs=1) as o2p,
                tc.tile_pool(name="o4p", bufs=2) as o4p,
                tc.tile_pool(name="psp", bufs=3, space="PSUM") as psp,
                tc.tile_pool(name="scrp", bufs=1, space="DRAM") as scrp,
            ):
                scr = scrp.tile([NST, SIS], bf16, tag="scr", name="scr")
                for si in range(NST):
                    h0 = si * SA
                    xb = xbp.tile([128, XBW], bf16, tag="xb", name="xb")
                    for wb in range(NWB):
                        w0 = wb * SB
                        blk = si * NWB + wb
                        ps = psp.tile([128, 1024], f32, tag="ps", name="ps")
                        for k in range(2):
                            lhsT = ZT[k][:, blk * 128:(blk + 1) * 128]
                            for half in range(2):
                                rhs = Z1P[k].rearrange("c (h w) -> c h w", h=HP)[
                                    :, h0 + 10 * half: h0 + 10 * (half + 1),
                                    w0:w0 + WINQ]
                                nc.tensor.matmul(
                                    ps[:, half * 512: half * 512 + 280], lhsT, rhs,
                                    start=(k == 0), stop=(k == 1))
                        # fused evacuation (both halves) with 1/sqrt(C) scale
                        esrc = bass.AP(ps.tensor, 0, [[1024, 128], [512, 2], [1, 280]])
                        edst = bass.AP(xb.tensor, wb * WBS, [[XBW, 128], [280, 2], [1, 280]])
                        if wb % 2 == 0:
                            nc.scalar.mul(edst, esrc, 1.0 / 16.0)
                        else:
                            nc.vector.tensor_scalar_mul(edst, esrc, 1.0 / 16.0)

                    # dense scratch write: 128 descriptors of 8960B
                    w_dst = bass.AP(scr.tensor, si * SIS, [[PIXS, 128], [1, XBW]])
                    nc.gpsimd.dma_start(w_dst, xb[:, :])

                # back-end: per w-half
                for wh in range(2):
                    o2 = o2p.tile([128, O2W], bf16, tag="o2", name="o2")
                    for dw in range(16):
                        rsrc = bass.AP(scr.tensor, PIXS * dw + WBS * 4 * wh,
                                       [[DHS, 128], [WBS, 4], [1, RUN]])
                        rdst = bass.AP(o2.tensor, dw * 4 * RUN,
                                       [[O2W, 128], [RUN, 4], [1, RUN]])
                        eng = nc.sync if dw % 2 == 0 else nc.scalar
                        eng.dma_start(rdst, rsrc)
                    for di in range(KS):
                        o4 = o4p.tile([128, O4W], bf16, tag="o4", name="o4")
                        csrc = bass.AP(o2.tensor, 28 * di,
                                       [[O2W, 128], [1, KS], [RUN, 4], [4 * RUN + 1, 16]])
                        cdst = bass.AP(o4.tensor, 0,
                                       [[O4W, 128], [64, KS], [16, 4], [1, 16]])
                        nc.vector.tensor_copy(cdst, csrc)
                        ow_dst = bass.AP(out_d, di * KS * H * W + wh * 64,
                                         [[W, 128], [H * W, KS], [1, 64]])
                        ow_src = bass.AP(o4.tensor, 0, [[O4W, 128], [64, KS], [1, 64]])
                        eng = nc.sync if di % 2 == 0 else nc.scalar
                        eng.dma_start(ow_dst, ow_src)

    nc.compile()
    return nc


def _get_nc():
    if "nc" not in _cache:
        _cache["nc"] = _build()
    return _cache["nc"]


def _prep(z_t: np.ndarray, z_t1: np.ndarray):
    """Host-side: cast to bf16, block-major reorder z_t, pad z_t1."""
    import ml_dtypes
    bf = ml_dtypes.bfloat16
    # zt block-major: free idx = ((si*8 + wb)*8 + dh)*16 + dw
    zt = z_t.astype(bf).reshape(2, 128, NST, SA, NWB, SB)
    zt = np.ascontiguousarray(zt.transpose(0, 1, 2, 4, 3, 5)).reshape(2, 128, H * W)
    z1 = np.pad(z_t1.astype(bf), ((0, 0), (RAD, RAD), (RAD, RAD)))
    z1 = np.ascontiguousarray(z1).reshape(2, 128, HP * WP)
    return zt, z1


def kernel(z_t: np.ndarray, z_t1: np.ndarray) -> np.ndarray:
    from concourse.bass_utils import run_bass_kernel_spmd

    nc = _get_nc()
    B = z_t.shape[0]
    in_maps = []
    for i in range(B):
        zt, z1 = _prep(z_t[i], z_t1[i])
        in_maps.append({"zt": zt, "z1p": z1})
    res = run_bass_kernel_spmd(nc, in_maps, core_ids=list(range(B)))
    return np.stack(
        [np.asarray(res.results[i]["out"]).astype(np.float32) for i in range(B)],
        axis=0)


# revision 5
# speedup vs baseline: 6.6667x; 1.3846x over previous
"""LocalCorrelation (13x13 cost volume) Trainium2 kernel.

Full inputs z_t, z_t1: [8, 256, 128, 128] f32 -> out [8, 169, 128, 128] f32.
out[b, 13*di+dj, h, w] = sum_c z_t[b,c,h,w] * pad(z_t1)[b,c,h+di,w+dj] / 16

Sharding: data-parallel over batch, 1 batch element per NeuronCore (8 cores).
Host pre-processing (free): cast to bf16, block-major reorder of z_t,
zero-pad of z_t1.  Host post-processing: upcast bf16 output to f32.

Per-core algorithm:
  Phase 1 (inputs resident):
  - 4 big input DMAs; ~40 warm-up matmuls keep/get PE to K=8/8 during load.
  - Per 8x16 pixel block: block-gram matmuls, stationary = z_t block
    [c, 128 pix], streaming = padded z_t1 20x28 window in two 280-col
    halves -> one [128,1024] PSUM tile (halves at cols 0 / 512),
    accumulated over the 2 c-chunks.
  - ONE fused PSUM->SBUF evacuation per block with the 1/16 scale.
  - ONE dense 2-dim DMA per stripe -> DRAM scratch:
      addr = 573664*si + 4480*pix + 560*wb + (28*p + q)
  Phase 2 (input pools closed; SBUF reused):
  - Superset gather, 16 DMAs with 728B runs (16K descriptors total):
      o2[h, (dw, wb, s)] = scr[71708*h + 4480*dw + 560*wb + s]
    71708 = 16*4480 + 28 makes the per-h start absorb the diagonal:
    s = 28*di + dw + dj.
  - Per di: one 4-dim strided copy -> o4[h, (dj, w)] (dj-shear is
    affine in free dims), one DMA -> out[di*13+dj, h, w] (256B runs).
"""

import numpy as np

C = 256
H = W = 128
KS = 13
KK = 169
RAD = 6
HP = WP = 140
SA = 8
SB = 16
NWB = 8
NST = 16
WINQ = 28
WIN = 560              # 20 * 28
PIXS = 4480            # scratch pixel stride (= 8*560, dense)
WBS = 560
DHS = 16 * PIXS + 28   # 71708
SIS = 8 * DHS          # 573664
XBW = 8 * WBS          # 4480
RUN = 364              # 13 * 28 superset run
O2W = 128 * RUN        # 46592
O4W = KS * W           # 1664

_cache = {}


def _build():
    import concourse.bass as bass
    import concourse.mybir as mybir
    import concourse.tile as tile
    from concourse import bacc

    f32 = mybir.dt.float32
    bf16 = mybir.dt.bfloat16

    nc = bacc.Bacc("TRN2", target_bir_lowering=False, debug=False)
    zt_d = nc.dram_tensor("zt", [2, 128, H * W], bf16, kind="ExternalInput")
    z1_d = nc.dram_tensor("z1p", [2, 128, HP * WP], bf16, kind="ExternalInput")
    out_d = nc.dram_tensor("out", [KK, H, W], bf16, kind="ExternalOutput")

    with tile.TileContext(nc) as tc:
        with tc.tile_pool(name="scrp", bufs=1, space="DRAM") as scrp:
            scr = scrp.tile([NST, SIS], bf16, tag="scr", name="scr")

            # ---------------- phase 1: correlation -> scratch ----------------
            with (
                tc.tile_pool(name="persist", bufs=1) as pp,
                tc.tile_pool(name="xbp", bufs=2) as xbp,
                tc.tile_pool(name="psp", bufs=3, space="PSUM") as psp,
                tc.tile_pool(name="wpp", bufs=1, space="PSUM") as wpp,
            ):
                ZT = [pp.tile([128, H * W], bf16, tag=f"zt{k}", name=f"zt{k}") for k in range(2)]
                Z1P = [pp.tile([128, HP * WP], bf16, tag=f"z1p{k}", name=f"z1p{k}") for k in range(2)]

                nc.sync.dma_start(Z1P[0][:, :], z1_d.ap()[0])
                nc.scalar.dma_start(ZT[0][:, :], zt_d.ap()[0])
                nc.sync.dma_start(Z1P[1][:, :], z1_d.ap()[1])
                nc.scalar.dma_start(ZT[1][:, :], zt_d.ap()[1])

                # PE warm-up while loads are in flight (keeps HAM at K=8/8)
                wt = pp.tile([128, 384], bf16, tag="wt", name="wt")
                nc.vector.memset(wt[:, :], 0.0)
                wps = wpp.tile([128, 256], f32, tag="wps", name="wps")
                for _ in range(40):
                    nc.tensor.matmul(wps[:, :], wt[:, 0:128], wt[:, 128:384],
                                     start=True, stop=True)

                for si in range(NST):
                    h0 = si * SA
                    xb = xbp.tile([128, XBW], bf16, tag="xb", name="xb")
                    for wb in range(NWB):
                        w0 = wb * SB
                        blk = si * NWB + wb
                        ps = psp.tile([128, 1024], f32, tag="ps", name="ps")
                        for k in range(2):
                            lhsT = ZT[k][:, blk * 128:(blk + 1) * 128]
                            for half in range(2):
                                rhs = Z1P[k].rearrange("c (h w) -> c h w", h=HP)[
                                    :, h0 + 10 * half: h0 + 10 * (half + 1),
                                    w0:w0 + WINQ]
                                nc.tensor.matmul(
                                    ps[:, half * 512: half * 512 + 280], lhsT, rhs,
                                    start=(k == 0), stop=(k == 1))
                        esrc = bass.AP(ps.tensor, 0, [[1024, 128], [512, 2], [1, 280]])
                        edst = bass.AP(xb.tensor, wb * WBS, [[XBW, 128], [280, 2], [1, 280]])
                        if wb % 2 == 0:
                            nc.scalar.mul(edst, esrc, 1.0 / 16.0)
                        else:
                            nc.vector.tensor_scalar_mul(edst, esrc, 1.0 / 16.0)

                    w_dst = bass.AP(scr.tensor, si * SIS, [[PIXS, 128], [1, XBW]])
                    nc.gpsimd.dma_start(w_dst, xb[:, :])

            # ---------------- phase 2: gather -> permute -> out ----------------
            with (
                tc.tile_pool(name="o2p", bufs=1) as o2p,
                tc.tile_pool(name="o4p", bufs=2) as o4p,
            ):
                o2 = o2p.tile([128, O2W], bf16, tag="o2", name="o2")
                for dw in range(16):
                    rsrc = bass.AP(scr.tensor, PIXS * dw, [[DHS, 128], [WBS, 8], [1, RUN]])
                    rdst = bass.AP(o2.tensor, dw * 8 * RUN, [[O2W, 128], [RUN, 8], [1, RUN]])
                    eng = nc.sync if dw % 2 == 0 else nc.scalar
                    eng.dma_start(rdst, rsrc)
                for di in range(KS):
                    o4 = o4p.tile([128, O4W], bf16, tag="o4", name="o4")
                    csrc = bass.AP(o2.tensor, 28 * di,
                                   [[O2W, 128], [1, KS], [RUN, 8], [8 * RUN + 1, 16]])
                    cdst = bass.AP(o4.tensor, 0,
                                   [[O4W, 128], [W, KS], [16, 8], [1, 16]])
                    if di % 2 == 0:
                        nc.vector.tensor_copy(cdst, csrc)
                    else:
                        nc.scalar.copy(cdst, csrc)
                    ow_dst = bass.AP(out_d, di * KS * H * W,
                                     [[W, 128], [H * W, KS], [1, W]])
                    ow_src = bass.AP(o4.tensor, 0, [[O4W, 128], [W, KS], [1, W]])
                    eng = nc.sync if di % 2 == 0 else nc.scalar
                    eng.dma_start(ow_dst, ow_src)

    nc.compile()
    return nc


def _get_nc():
    if "nc" not in _cache:
        _cache["nc"] = _build()
    return _cache["nc"]


def _prep(z_t: np.ndarray, z_t1: np.ndarray):
    """Host-side: cast to bf16, block-major reorder z_t, pad z_t1."""
    import ml_dtypes
    bf = ml_dtypes.bfloat16
    zt = z_t.astype(bf).reshape(2, 128, NST, SA, NWB, SB)
    zt = np.ascontiguousarray(zt.transpose(0, 1, 2, 4, 3, 5)).reshape(2, 128, H * W)
    z1 = np.pad(z_t1.astype(bf), ((0, 0), (RAD, RAD), (RAD, RAD)))
    z1 = np.ascontiguousarray(z1).reshape(2, 128, HP * WP)
    return zt, z1


def kernel(z_t: np.ndarray, z_t1: np.ndarray) -> np.ndarray:
    from concourse.bass_utils import run_bass_kernel_spmd

    nc = _get_nc()
    B = z_t.shape[0]
    in_maps = []
    for i in range(B):
        zt, z1 = _prep(z_t[i], z_t1[i])
        in_maps.append({"zt": zt, "z1p": z1})
    res = run_bass_kernel_spmd(nc, in_maps, core_ids=list(range(B)))
    return np.stack(
        [np.asarray(res.results[i]["out"]).astype(np.float32) for i in range(B)],
        axis=0)


# revision 6
# speedup vs baseline: 6.7494x; 1.0124x over previous
"""LocalCorrelation (13x13 cost volume) Trainium2 kernel.

Full inputs z_t, z_t1: [8, 256, 128, 128] f32 -> out [8, 169, 128, 128] f32.
out[b, 13*di+dj, h, w] = sum_c z_t[b,c,h,w] * pad(z_t1)[b,c,h+di,w+dj] / 16

Sharding: data-parallel over batch, 1 batch element per NeuronCore (8 cores).
Host pre-processing (free): cast to bf16, block-major reorder of z_t,
zero-pad of z_t1.  Host post-processing: upcast bf16 output to f32.

Per-core algorithm:
  Phase 1 (inputs resident):
  - 4 big input DMAs; ~40 warm-up matmuls keep/get PE to K=8/8 during load.
  - Per 8x16 pixel block: block-gram matmuls, stationary = z_t block
    [c, 128 pix], streaming = padded z_t1 20x28 window in two 280-col
    halves -> one [128,1024] PSUM tile (halves at cols 0 / 512),
    accumulated over the 2 c-chunks.
  - ONE fused PSUM->SBUF evacuation per block with the 1/16 scale.
  - ONE dense 2-dim DMA per stripe -> DRAM scratch:
      addr = 573664*si + 4480*pix + 560*wb + (28*p + q)
  Phase 2 (input pools closed; SBUF reused):
  - Superset gather, 16 DMAs with 728B runs (16K descriptors total):
      o2[h, (dw, wb, s)] = scr[71708*h + 4480*dw + 560*wb + s]
    71708 = 16*4480 + 28 makes the per-h start absorb the diagonal:
    s = 28*di + dw + dj.
  - Per di: one 4-dim strided copy -> o4[h, (dj, w)] (dj-shear is
    affine in free dims), one DMA -> out[di*13+dj, h, w] (256B runs).
"""

import numpy as np

C = 256
H = W = 128
KS = 13
KK = 169
RAD = 6
HP = WP = 140
SA = 8
SB = 16
NWB = 8
NST = 16
WINQ = 28
WIN = 560              # 20 * 28
PIXS = 4480            # scratch pixel stride (= 8*560, dense)
WBS = 560
DHS = 16 * PIXS + 28   # 71708
SIS = 8 * DHS          # 573664
XBW = 8 * WBS          # 4480
RUN = 364              # 13 * 28 superset run
O2W = 128 * RUN        # 46592
O4W = KS * W           # 1664

_cache = {}


def _build():
    import concourse.bass as bass
    import concourse.mybir as mybir
    import concourse.tile as tile
    from concourse import bacc

    f32 = mybir.dt.float32
    bf16 = mybir.dt.bfloat16

    nc = bacc.Bacc("TRN2", target_bir_lowering=False, debug=False)
    zt_d = nc.dram_tensor("zt", [2, 128, H * W], bf16, kind="ExternalInput")
    z1_d = nc.dram_tensor("z1p", [2, 128, HP * WP], bf16, kind="ExternalInput")
    out_d = nc.dram_tensor("out", [KK, H, W], bf16, kind="ExternalOutput")

    with tile.TileContext(nc) as tc:
        with tc.tile_pool(name="scrp", bufs=1, space="DRAM") as scrp:
            scr = scrp.tile([NST, SIS], bf16, tag="scr", name="scr")

            # ---------------- phase 1: correlation -> scratch ----------------
            with (
                tc.tile_pool(name="persist", bufs=1) as pp,
                tc.tile_pool(name="xbp", bufs=2) as xbp,
                tc.tile_pool(name="psp", bufs=4, space="PSUM") as psp,
            ):
                ZT = [pp.tile([128, H * W], bf16, tag=f"zt{k}", name=f"zt{k}") for k in range(2)]
                Z1P = [pp.tile([128, HP * WP], bf16, tag=f"z1p{k}", name=f"z1p{k}") for k in range(2)]

                nc.sync.dma_start(Z1P[0][:, :], z1_d.ap()[0])
                nc.scalar.dma_start(ZT[0][:, :], zt_d.ap()[0])
                nc.sync.dma_start(Z1P[1][:, :], z1_d.ap()[1])
                nc.scalar.dma_start(ZT[1][:, :], zt_d.ap()[1])

                # PE warm-up while loads are in flight (keeps HAM at K=8/8)
                wt = pp.tile([128, 640], bf16, tag="wt", name="wt")
                nc.vector.memset(wt[:, :], 0.0)
                for _ in range(100):
                    wps = psp.tile([128, 1024], f32, tag="ps", name="ps")
                    nc.tensor.matmul(wps[:, 0:512], wt[:, 0:128], wt[:, 128:640],
                                     start=True, stop=True)

                for si in range(NST):
                    h0 = si * SA
                    xb = xbp.tile([128, XBW], bf16, tag="xb", name="xb")
                    for wb in range(NWB):
                        w0 = wb * SB
                        blk = si * NWB + wb
                        ps = psp.tile([128, 1024], f32, tag="ps", name="ps")
                        for k in range(2):
                            lhsT = ZT[k][:, blk * 128:(blk + 1) * 128]
                            for half in range(2):
                                rhs = Z1P[k].rearrange("c (h w) -> c h w", h=HP)[
                                    :, h0 + 10 * half: h0 + 10 * (half + 1),
                                    w0:w0 + WINQ]
                                nc.tensor.matmul(
                                    ps[:, half * 512: half * 512 + 280], lhsT, rhs,
                                    start=(k == 0), stop=(k == 1))
                        esrc = bass.AP(ps.tensor, 0, [[1024, 128], [512, 2], [1, 280]])
                        edst = bass.AP(xb.tensor, wb * WBS, [[XBW, 128], [280, 2], [1, 280]])
                        if wb % 2 == 0:
                            nc.scalar.mul(edst, esrc, 1.0 / 16.0)
                        else:
                            nc.vector.tensor_scalar_mul(edst, esrc, 1.0 / 16.0)

                    w_dst = bass.AP(scr.tensor, si * SIS, [[PIXS, 128], [1, XBW]])
                    nc.gpsimd.dma_start(w_dst, xb[:, :])

            # ---------------- phase 2: gather -> permute -> out ----------------
            with (
                tc.tile_pool(name="o2p", bufs=1) as o2p,
                tc.tile_pool(name="o4p", bufs=2) as o4p,
            ):
                o2 = o2p.tile([128, O2W], bf16, tag="o2", name="o2")
                for dw in range(16):
                    rsrc = bass.AP(scr.tensor, PIXS * dw, [[DHS, 128], [WBS, 8], [1, RUN]])
                    rdst = bass.AP(o2.tensor, dw * 8 * RUN, [[O2W, 128], [RUN, 8], [1, RUN]])
                    eng = nc.sync if dw % 2 == 0 else nc.scalar
                    eng.dma_start(rdst, rsrc)
                for di in range(KS):
                    o4 = o4p.tile([128, O4W], bf16, tag="o4", name="o4")
                    csrc = bass.AP(o2.tensor, 28 * di,
                                   [[O2W, 128], [1, KS], [RUN, 8], [8 * RUN + 1, 16]])
                    cdst = bass.AP(o4.tensor, 0,
                                   [[O4W, 128], [W, KS], [16, 8], [1, 16]])
                    if di % 3 == 0:
                        nc.vector.tensor_copy(cdst, csrc)
                    elif di % 3 == 1:
                        nc.scalar.copy(cdst, csrc)
                    else:
                        nc.gpsimd.tensor_copy(cdst, csrc)
                    ow_dst = bass.AP(out_d, di * KS * H * W,
                                     [[W, 128], [H * W, KS], [1, W]])
                    ow_src = bass.AP(o4.tensor, 0, [[O4W, 128], [W, KS], [1, W]])
                    eng = nc.sync if di % 2 == 0 else nc.scalar
                    eng.dma_start(ow_dst, ow_src)

    nc.compile()
    return nc


def _get_nc():
    if "nc" not in _cache:
        _cache["nc"] = _build()
    return _cache["nc"]


def _prep(z_t: np.ndarray, z_t1: np.ndarray):
    """Host-side: cast to bf16, block-major reorder z_t, pad z_t1."""
    import ml_dtypes
    bf = ml_dtypes.bfloat16
    zt = z_t.astype(bf).reshape(2, 128, NST, SA, NWB, SB)
    zt = np.ascontiguousarray(zt.transpose(0, 1, 2, 4, 3, 5)).reshape(2, 128, H * W)
    z1 = np.pad(z_t1.astype(bf), ((0, 0), (RAD, RAD), (RAD, RAD)))
    z1 = np.ascontiguousarray(z1).reshape(2, 128, HP * WP)
    return zt, z1


def kernel(z_t: np.ndarray, z_t1: np.ndarray) -> np.ndarray:
    from concourse.bass_utils import run_bass_kernel_spmd

    nc = _get_nc()
    B = z_t.shape[0]
    in_maps = []
    for i in range(B):
        zt, z1 = _prep(z_t[i], z_t1[i])
        in_maps.append({"zt": zt, "z1p": z1})
    res = run_bass_kernel_spmd(nc, in_maps, core_ids=list(range(B)))
    return np.stack(
        [np.asarray(res.results[i]["out"]).astype(np.float32) for i in range(B)],
        axis=0)
